# revision 39
# baseline (speedup 1.0000x reference)
"""Trainium2 Bass kernel for the BAF bilinear-attention-fusion module.

Sharding: one bilinear chunk c per NeuronCore (C=8 chunks, 8 cores).
Per layer each core computes its chunk's (B, Lq, Lk) score contribution
numer_c * rsqrt(norm_c); a ReduceScatter (f16) sums chunks and hands each
core a 64-query slice for softmax+attend; an AllGather rebuilds the
attended tensor for the (replicated) scramble + output projection + BN.

z post-processing uses the signed-sqrt identity p = sign(z)*sqrt(|z|)
computed as (z16 & 0x8000) | sqrt(|z16|): one Sqrt LUT pass on the Scalar
engine plus int16-bitcast DVE ops that hit the 4x all-SBUF fp16 perf mode.
The chunk norm rhs is |z16| itself (== p^2). PSUM-exit copies alternate
DVE/ACT to balance load. PE accumulates numer (w_s-scaled identities) and
norm (identity) per s-slice, software-pipelined two waves behind the z
matmuls. Key-side projections (xi -> keyT -> x0T -> a-factors) are
layer-invariant and computed once.
"""
import sys
sys.path.insert(0, '/opt/trn_rl_repo')

import numpy as np
import ml_dtypes  # noqa: F401

import concourse.bass as bass  # noqa: F401
import concourse.tile as tile
from concourse import bacc, mybir
from concourse.bass_utils import run_bass_kernel_spmd

# Restrict the activation-table universe to the two sets this kernel needs
# (sqrt for the z chain + BN, exp for softmax); the greedy act-table-load
# pass then emits the minimal ~5 table switches instead of ping-ponging.
from concourse import bacc as _bacc_mod
_orig_gat = _bacc_mod.get_activation_tables
_KEEP_TABLES = ("sqrt_and_others", "natural_log_exp_and_others")


def _forced_tables(arch):
    t = _orig_gat(arch)
    return {name: (fs if name in _KEEP_TABLES else set())
            for name, fs in t.items()}


_bacc_mod.get_activation_tables = _forced_tables

dt = mybir.dt
AF = mybir.ActivationFunctionType
ALU = mybir.AluOpType
f32 = dt.float32
f32r = dt.float32r
f16 = dt.float16
i16 = dt.int16

B, L, D, H, C, R = 2, 256, 256, 512, 8, 5
S = H // C            # 64
RP = 8                # rank padded to 8 (z-matmul K dim)
BL = B * L            # 512
NCORES = 8

_cache = {}


def build_bass():
    nc = bacc.Bacc("TRN2", target_bir_lowering=False, debug=False,
                   num_devices=NCORES)

    def register_const(value, dtype=f32):
        t = nc.alloc_sbuf_tensor(f"const-{dtype.name}-{value}", [128, 1], dtype)
        nc.gpsimd.memset(t.ap(), value)
        nc.const_aps.aps[(dtype, value)] = t.ap()

    register_const(1e-12)
    register_const(1e-5)
    nc.all_engine_barrier()

    def din(name, shape, dtype=f32r):
        return nc.dram_tensor(name, shape, dtype, kind="ExternalInput")

    dd = {}
    dd["xiT"] = din("xiT", [D, BL])
    dd["xtT"] = din("xtT", [D, BL])
    dd["Wv"] = din("Wv", [D, H]); dd["bvr"] = din("bvr", [1, H])
    dd["Wk"] = din("Wk", [D, H]); dd["bkp"] = din("bkp", [128, 4], f32)
    dd["Wq"] = din("Wq", [D, H]); dd["bqp"] = din("bqp", [128, 4], f32)
    dd["Wm"] = din("Wm", [H, D], f16); dd["bmp"] = din("bmp", [128, 2], f32)
    dd["W0c"] = din("W0c", [H, S]); dd["b0c"] = din("b0c", [S, 1], f32)
    dd["W1c"] = din("W1c", [H, S]); dd["b1c"] = din("b1c", [S, 1], f32)
    dd["M0p"] = din("M0p", [S, S * RP]); dd["Mb0p"] = din("Mb0p", [128, 4], f32)
    dd["M1p"] = din("M1p", [S, S * RP]); dd["Mb1p"] = din("Mb1p", [128, 4], f32)
    dd["eyeW"] = din("eyeW", [128, S * 128], f16)
    dd["eye16"] = din("eye16", [128, 128], f16)
    dd["eye32"] = din("eye32", [128, 128], f32)
    dd["ones1"] = din("ones1", [1, 128])
    dd["ones128"] = din("ones128", [128, 1])
    dd["sel"] = din("sel", [BL, L])
    dd["gam"] = din("gam", [1, D], f32)
    dd["bet"] = din("bet", [1, D], f32)
    dd["out"] = nc.dram_tensor("vT_out", [D, BL], f32r, kind="ExternalOutput")

    with tile.TileContext(nc) as tc:
        _program(nc, tc, dd)
    nc.finalize()
    return nc


def _program(nc, tc, dd):
    with tc.tile_pool(name="const", bufs=1) as cp, \
         tc.tile_pool(name="dram", bufs=1, space="DRAM") as dram:

        def load(dram_t, shape, dtype=f32r, tag=None):
            t = cp.tile(shape, dtype, tag=tag or dram_t.name)
            nc.sync.dma_start(t[:], dram_t[:])
            return t

        def load4(dram_t, rows, cols, n, dtype=f32r):
            return [load(dram_t[kt * 128:(kt + 1) * 128, :], [rows, cols],
                         dtype, tag=f"{dram_t.name}{kt}") for kt in range(n)]

        # load order = consumption order: key-side precompute deps first,
        # then query-side layer-1 deps, z-phase tables, then output-phase.
        xiT = load4(dd["xiT"], 128, BL, 2)
        Wk = load4(dd["Wk"], 128, H, 2)
        bkp = load(dd["bkp"], [128, 4], f32)
        W0c = load4(dd["W0c"], 128, S, 4)
        b0c = load(dd["b0c"], [S, 1], f32)
        M0p = load(dd["M0p"], [S, S * RP])
        Mb0p = load(dd["Mb0p"], [128, 4], f32)
        xtT = load4(dd["xtT"], 128, BL, 2)
        Wq = load4(dd["Wq"], 128, H, 2)
        bqp = load(dd["bqp"], [128, 4], f32)
        W1c = load4(dd["W1c"], 128, S, 4)
        b1c = load(dd["b1c"], [S, 1], f32)
        M1p = load(dd["M1p"], [S, S * RP])
        Mb1p = load(dd["Mb1p"], [128, 4], f32)
        eye16 = load(dd["eye16"], [128, 128], f16)
        eyeW = load(dd["eyeW"], [128, S * 128], f16)
        Wv = load4(dd["Wv"], 128, H, 2)
        bvr = load(dd["bvr"], [1, H])
        sel = load4(dd["sel"], 128, L, 4)
        eye32 = load(dd["eye32"], [128, 128], f32)
        ones1 = load(dd["ones1"], [1, 128])
        ones128 = load(dd["ones128"], [128, 1])
        Wm = load4(dd["Wm"], 128, D, 4, f16)
        bmp = load(dd["bmp"], [128, 2], f32)
        gam = load(dd["gam"], [1, D], f32)
        bet = load(dd["bet"], [1, D], f32)

        w = dict(cp=cp, xiT=xiT, Wv=Wv, Wq=Wq, Wm=Wm, W1c=W1c,
                 sel=sel, M1p=M1p, eyeW=eyeW, eye16=eye16,
                 eye32=eye32, ones1=ones1, ones128=ones128, bvr=bvr,
                 bqp=bqp, bmp=bmp, b1c=b1c, Mb1p=Mb1p, gam=gam, bet=bet)

        # ---- key-side precompute (layer-invariant: depends only on xi) ----
        aF = dram.tile([S * RP, BL], f16, tag="aF")
        with tc.tile_pool(name="kps", bufs=2, space="PSUM") as kps, \
             tc.tile_pool(name="ktmp", bufs=1) as ktmp:
            keyT = []
            for mt in range(4):
                p = kps.tile([128, BL], f32, tag="kproj")
                for kt in range(2):
                    nc.tensor.matmul(p[:], Wk[kt][:, mt * 128:(mt + 1) * 128],
                                     xiT[kt][:], start=(kt == 0), stop=(kt == 1))
                t = ktmp.tile([128, BL], f32r, tag=f"keyT{mt}")
                nc.scalar.activation(t[:], p[:], AF.Identity,
                                     bias=bkp[:, mt:mt + 1])
                keyT.append(t)
            p = kps.tile([S, BL], f32, tag="kprojS")
            for kt in range(4):
                nc.tensor.matmul(p[:], W0c[kt][:], keyT[kt][:],
                                 start=(kt == 0), stop=(kt == 3))
            x0T = ktmp.tile([S, BL], f32r, tag="x0T")
            nc.scalar.activation(x0T[:], p[:], AF.Identity, bias=b0c[:])
            for mt in range(4):
                p = kps.tile([128, BL], f32, tag="kproj")
                nc.tensor.matmul(p[:], M0p[:, mt * 128:(mt + 1) * 128],
                                 x0T[:], start=True, stop=True)
                t = ktmp.tile([128, BL], f16, tag=f"aT{mt}")
                nc.scalar.activation(t[:], p[:], AF.Identity,
                                     bias=Mb0p[:, mt:mt + 1])
                nc.sync.dma_start(aF[mt * 128:(mt + 1) * 128, :], t[:])
        w["aFv"] = aF[:].rearrange("(s r) bl -> r s bl", r=RP)

        vT = xtT
        for layer in range(2):
            vT = _layer(nc, tc, dram, layer, vT, w)

        for kt in range(2):
            nc.sync.dma_start(dd["out"][kt * 128:(kt + 1) * 128, :], vT[kt][:])


def _layer(nc, tc, dram, layer, vT, w):
    X = mybir.AxisListType.X
    RG = [list(range(NCORES))]
    L_ = layer
    bqp, b1c, Mb1p = w["bqp"], w["b1c"], w["Mb1p"]

    with tc.tile_pool(name=f"l{L_}sb", bufs=1) as sb:
        # ---------------- query-side projections ----------------
        bbF = dram.tile([S * RP, BL], f16, tag=f"bbF{L_}")
        with tc.tile_pool(name=f"l{L_}pps", bufs=2, space="PSUM") as ps:
            with tc.tile_pool(name=f"l{L_}ptmp", bufs=1) as ptmp:
                queryT = []
                for mt in range(4):
                    p = ps.tile([128, BL], f32, tag="proj")
                    for kt in range(2):
                        nc.tensor.matmul(p[:],
                                         w["Wq"][kt][:, mt * 128:(mt + 1) * 128],
                                         vT[kt][:],
                                         start=(kt == 0), stop=(kt == 1))
                    t = ptmp.tile([128, BL], f32r, tag=f"queryT{mt}")
                    nc.scalar.activation(t[:], p[:], AF.Identity,
                                         bias=bqp[:, mt:mt + 1])
                    queryT.append(t)

                p = ps.tile([S, BL], f32, tag="projS")
                for kt in range(4):
                    nc.tensor.matmul(p[:], w["W1c"][kt][:], queryT[kt][:],
                                     start=(kt == 0), stop=(kt == 3))
                x1T = sb.tile([S, BL], f32r, tag="x1T")
                nc.scalar.activation(x1T[:], p[:], AF.Identity, bias=b1c[:])

            for mt in range(4):
                p = ps.tile([128, BL], f32, tag="proj")
                nc.tensor.matmul(p[:], w["M1p"][:, mt * 128:(mt + 1) * 128],
                                 x1T[:], start=True, stop=True)
                t = sb.tile([128, BL], f16, tag=f"bT{mt}")
                nc.scalar.activation(t[:], p[:], AF.Identity,
                                     bias=Mb1p[:, mt:mt + 1])
                nc.sync.dma_start(bbF[mt * 128:(mt + 1) * 128, :], t[:])
        bbFv = bbF[:].rearrange("(s r) bl -> r s bl", r=RP)
        aFv = w["aFv"]

        # ---------------- bilinear z phase ----------------
        # one contrib slab + ReduceScatter per (b,ih) block: RS#k overlaps
        # the z compute of later blocks; core c owns score rows
        # {128k + 16c + t}, i.e. 16 rows of each block.
        contribs = dram.tile([BL, L], f16, tag="contribs")
        with tc.tile_pool(name=f"l{L_}zw", bufs=3, space="PSUM") as zp, \
             tc.tile_pool(name=f"l{L_}acc", bufs=1, space="PSUM") as accp, \
             tc.tile_pool(name=f"l{L_}zsb", bufs=5) as zsb, \
             tc.tile_pool(name=f"l{L_}zsb2", bufs=2) as zsb2, \
             tc.tile_pool(name=f"l{L_}fac", bufs=2) as fac:

            def emit_accums(item):
                wv, pw_t, a16_t, numer_t, norm_t = item
                for sl in range(4):
                    s = 4 * wv + sl
                    nc.tensor.matmul(numer_t[:],
                                     w["eyeW"][:, s * 128:(s + 1) * 128],
                                     pw_t[:, sl * L:(sl + 1) * L],
                                     start=(s == 0), stop=(s == 63))
                    nc.tensor.matmul(norm_t[:], w["eye16"][:],
                                     a16_t[:, sl * L:(sl + 1) * L],
                                     start=(s == 0), stop=(s == 63))

            def load_group(g, b, i0, store):
                ta = fac.tile([RP, 16 * L], f16, tag=f"awt{g % 2}")
                nc.sync.dma_start(
                    ta[:].rearrange("r (s j) -> r s j", s=16),
                    aFv[:, g * 16:(g + 1) * 16, b * L:(b + 1) * L])
                tb = fac.tile([RP, 16 * 128], f16, tag=f"bwt{g % 2}")
                nc.sync.dma_start(
                    tb[:].rearrange("r (s i) -> r s i", s=16),
                    bbFv[:, g * 16:(g + 1) * 16, i0:i0 + 128])
                store[g] = (ta, tb)

            for b in range(B):
                for ih in range(2):
                    i0 = b * L + ih * 128
                    numer = accp.tile([128, L], f32, tag="numer")
                    norm = accp.tile([128, L], f32, tag="norm")
                    grp = {}
                    load_group(0, b, i0, grp)
                    load_group(1, b, i0, grp)
                    queue = []
                    for wv in range(16):
                        g, part = wv // 4, wv % 4
                        if part == 0 and g + 2 < 4:
                            load_group(g + 2, b, i0, grp)
                        awt, bwt = grp[g]
                        zw = zp.tile([128, 4 * L], f32, tag="zw")
                        for sl in range(4):
                            k = part * 4 + sl
                            nc.tensor.matmul(
                                zw[:, sl * L:(sl + 1) * L],
                                bwt[:, k * 128:(k + 1) * 128],
                                awt[:, k * L:(k + 1) * L],
                                start=True, stop=True)
                        # PSUM-exit copies alternate ACT/DVE. The f16 pairs
                        # are bitcast int32 (bitwise ops are DVE-only and
                        # 32-bit-only on this hw): abs masks both halves'
                        # sign bits; the sign-merge is one fused
                        # (z & 0x80008000) + sq -- fp16 magnitudes stay
                        # below 0x8000 so the add cannot carry.
                        z16 = zsb2.tile([128, 4 * L], f16, tag="z16")
                        if wv % 16 in (1, 3, 5, 7, 9, 11, 13):
                            nc.vector.tensor_copy(z16[:], zw[:])
                        else:
                            nc.scalar.activation(z16[:], zw[:], AF.Identity,
                                                 bias=0.0)
                        a16 = zsb.tile([128, 4 * L], f16, tag="a16")
                        nc.vector.tensor_scalar(
                            a16[:].bitcast(dt.int32), z16[:].bitcast(dt.int32),
                            0x7FFF7FFF, None, ALU.bitwise_and)
                        sq = zsb2.tile([128, 4 * L], f16, tag="sq")
                        nc.scalar.activation(sq[:], a16[:], AF.Sqrt, bias=0.0)
                        pw = zsb.tile([128, 4 * L], f16, tag="pw")
                        nc.vector.scalar_tensor_tensor(
                            pw[:].bitcast(dt.int32), z16[:].bitcast(dt.int32),
                            0x80008000 - (1 << 32), sq[:].bitcast(dt.int32),
                            ALU.bitwise_and, ALU.add)
                        queue.append((wv, pw, a16, numer, norm))
                        if len(queue) >= 5:
                            emit_accums(queue.pop(0))
                    for item in queue:
                        emit_accums(item)
                    queue = []
                    sqn = zsb2.tile([128, L], f32, tag="sqn")
                    nc.scalar.activation(sqn[:], norm[:], AF.Sqrt, bias=1e-12)
                    rstd = zsb2.tile([128, L], f32, tag="rstd")
                    nc.vector.reciprocal(rstd[:], sqn[:])
                    contrib = zsb2.tile([128, L], f16, tag="contrib")
                    nc.vector.scalar_tensor_tensor(
                        contrib[:], numer[:], 1.0, rstd[:], ALU.mult, ALU.mult)
                    nc.sync.dma_start(contribs[i0:i0 + 128, :], contrib[:])

        # ---------- ReduceScatter scores (f16) ----------
        scores_d = dram.tile([S, L], f16, tag="scores")
        nc.gpsimd.collective_compute("ReduceScatter", ALU.add,
                                     ins=[contribs.opt()],
                                     outs=[scores_d.opt()],
                                     replica_groups=RG)

        # ---------- value projection (overlaps the RS chain) ----------
        # value tiles are copied out of PSUM directly in h-column e-order
        # (evens then odds) so they serve as the attend rhs per batch and
        # the scramble gather stays contiguous.
        atted_d = dram.tile([S, H], f16, tag="atted")
        with tc.tile_pool(name=f"l{L_}aps", bufs=2, space="PSUM") as aps:
            vsel = []
            value = []
            for mt in range(4):
                p = aps.tile([128, H], f32, tag="vproj")
                for kt in range(2):
                    nc.tensor.matmul(p[:], vT[kt][:, mt * 128:(mt + 1) * 128],
                                     w["Wv"][kt][:], start=(kt == 0), stop=False)
                nc.tensor.matmul(p[:], w["ones1"][:, 0:128], w["bvr"][:],
                                 start=False, stop=True)
                t = sb.tile([128, H], f32r, tag=f"value{mt}")
                nc.vector.tensor_copy(t[:], p[:])
                value.append(t)

            # value rows for this core's batch, h-columns permuted to
            # e-order (evens then odds) so the scramble gather is contiguous
            vsel = []
            for jt in range(2):
                p = aps.tile([128, H], f32, tag="vsel")
                for kt in range(4):
                    nc.tensor.matmul(p[:],
                                     w["sel"][kt][:, jt * 128:(jt + 1) * 128],
                                     value[kt][:],
                                     start=(kt == 0), stop=(kt == 3))
                t = sb.tile([128, H], f32r, tag=f"vsel{jt}")
                nc.vector.tensor_copy(
                    t[:].rearrange("j (ih iq) -> j ih iq", ih=2),
                    p[:].rearrange("j (iq ih) -> j ih iq", ih=2))
                vsel.append(t)

            # ---------- softmax + attend ----------
            # scores are bounded (|s| < ~30) so exp cannot overflow fp32:
            # skip the max-subtraction for a shorter serial chain.
            sc16 = sb.tile([S, L], f16, tag="sc16")
            nc.sync.dma_start(sc16[:], scores_d[:])
            # attend on unnormalized exp scores; the softmax denominator is
            # folded into the atted PSUM-exit as a per-row ACT scale.
            ex = sb.tile([S, L], f32, tag="ex")
            sume = sb.tile([S, 1], f32, tag="sume")
            nc.scalar.activation(ex[:], sc16[:], AF.Exp, bias=0.0,
                                 accum_out=sume[:])
            rcp = sb.tile([S, 1], f32, tag="rcp")
            nc.vector.reciprocal(rcp[:], sume[:])

            attT = []
            for jt in range(2):
                p = aps.tile([128, S], f32, tag="attT")
                nc.tensor.transpose(p[:], ex[:, jt * 128:(jt + 1) * 128],
                                    w["eye32"][0:S, 0:S])
                t = sb.tile([128, S], f32r, tag=f"attT{jt}")
                nc.vector.tensor_copy(t[:], p[:])
                attT.append(t)

            pa = aps.tile([S, H], f32, tag="atted")
            for jt in range(2):
                nc.tensor.matmul(pa[:], attT[jt][:], vsel[jt][:],
                                 start=(jt == 0), stop=(jt == 1))
            ats = sb.tile([S, H], f16, tag="atteds")
            nc.scalar.activation(ats[:], pa[:], AF.Identity, bias=0.0,
                                 scale=rcp[:])
            nc.sync.dma_start(atted_d[:], ats[:])

        # ---------- AllGather (f16 payload); scramble; Wm; BN ----------
        attedall = dram.tile([BL, H], f16, tag="attedall")
        nc.gpsimd.collective_compute("AllGather", ALU.bypass,
                                     ins=[atted_d.opt()],
                                     outs=[attedall.opt()],
                                     replica_groups=RG)

        with tc.tile_pool(name=f"l{L_}fps", bufs=1, space="PSUM") as fps:
            # scrT[j', (b,i')] = atted_perm[b*256 + j'%256, (j'//256)*256 + i']
            aa = attedall[:].rearrange("(b lh p) (ih iq) -> lh ih p b iq",
                                       b=2, lh=2, ih=2)
            scrT = []
            for jt in range(4):
                t = sb.tile([128, BL], f16, tag=f"scrT{jt}")
                nc.sync.dma_start(
                    t[:].rearrange("p (b iq) -> p b iq", b=2),
                    aa[jt % 2, jt // 2])
                scrT.append(t)

            attnT = []
            for mt in range(2):
                p = fps.tile([128, BL], f32, tag="attnT")
                for kt in range(4):
                    nc.tensor.matmul(p[:],
                                     w["Wm"][kt][:, mt * 128:(mt + 1) * 128],
                                     scrT[kt][:],
                                     start=(kt == 0), stop=(kt == 3))
                t = sb.tile([128, BL], f32r, tag=f"attnT{mt}")
                nc.scalar.activation(t[:], p[:], AF.Identity,
                                     bias=w["bmp"][:, mt:mt + 1])
                attnT.append(t)

            cs = fps.tile([1, BL], f32, tag="cs")
            for mt in range(2):
                nc.tensor.matmul(cs[:], w["ones128"][:], attnT[mt][:],
                                 start=(mt == 0), stop=(mt == 1))
            cs2 = fps.tile([1, BL], f32, tag="cs2")
            sqs = []
            for mt in range(2):
                sq = sb.tile([128, BL], f32r, tag=f"sq{mt}")
                nc.scalar.activation(sq[:], attnT[mt][:].bitcast(f32),
                                     AF.Square)
                sqs.append(sq)
            for mt in range(2):
                nc.tensor.matmul(cs2[:], w["ones128"][:], sqs[mt][:],
                                 start=(mt == 0), stop=(mt == 1))

            def stt(out, in0, scalar, in1, op0, op1):
                nc.vector.scalar_tensor_tensor(out, in0, scalar, in1, op0, op1)

            # per-position sums read straight from PSUM; 1/BL folds into
            # the mu scale and the Sqrt input scale.
            csum = sb.tile([1, D], f32, tag="csum")
            nc.vector.tensor_tensor(csum[:], cs[:, 0:D], cs[:, D:BL], ALU.add)
            c2sum = sb.tile([1, D], f32, tag="c2sum")
            nc.vector.tensor_tensor(c2sum[:], cs2[:, 0:D], cs2[:, D:BL],
                                    ALU.add)
            mu = sb.tile([1, D], f32, tag="mu")
            nc.vector.tensor_scalar_mul(mu[:], csum[:], 1.0 / BL)
            varBL = sb.tile([1, D], f32, tag="varBL")
            stt(varBL[:], mu[:], -1.0, csum[:], ALU.mult, ALU.mult)
            stt(varBL[:], varBL[:], 1.0, c2sum[:], ALU.mult, ALU.add)
            sqv = sb.tile([1, D], f32, tag="sqv")
            nc.scalar.activation(sqv[:], varBL[:], AF.Sqrt, bias=1e-5,
                                 scale=1.0 / BL)
            rstd = sb.tile([1, D], f32, tag="rstdb")
            nc.vector.reciprocal(rstd[:], sqv[:])
            Am = sb.tile([1, D], f32, tag="Am")
            stt(Am[:], w["gam"][:], 1.0, rstd[:], ALU.mult, ALU.mult)
            Bb = sb.tile([1, D], f32, tag="Bb")
            stt(Bb[:], mu[:], -1.0, Am[:], ALU.mult, ALU.mult)
            stt(Bb[:], Bb[:], 1.0, w["bet"][:], ALU.mult, ALU.add)
            A2 = sb.tile([1, BL], f32r, tag="A2")
            nc.vector.tensor_copy(A2[:, 0:D], Am[:])
            nc.vector.tensor_copy(A2[:, D:BL], Am[:])
            B2 = sb.tile([1, BL], f32r, tag="B2")
            nc.vector.tensor_copy(B2[:, 0:D], Bb[:])
            nc.vector.tensor_copy(B2[:, D:BL], Bb[:])
            Abc = fps.tile([128, BL], f32, tag="Abc")
            nc.tensor.matmul(Abc[:], w["ones1"][:, 0:128], A2[:],
                             start=True, stop=True)
            Bbc = fps.tile([128, BL], f32, tag="Bbc")
            nc.tensor.matmul(Bbc[:], w["ones1"][:, 0:128], B2[:],
                             start=True, stop=True)

            vnew = []
            for mt in range(2):
                t1 = sb.tile([128, BL], f32, tag="t1")
                stt(t1[:], attnT[mt][:].bitcast(f32), 1.0, Abc[:],
                    ALU.mult, ALU.mult)
                stt(t1[:], t1[:], 1.0, Bbc[:], ALU.mult, ALU.add)
                nc.vector.tensor_scalar_max(t1[:], t1[:], 0.0)
                t2 = w["cp"].tile([128, BL], f32r, tag=f"vnew{L_}{mt}")
                stt(t2[:], t1[:], 1.0, vT[mt][:].bitcast(f32), ALU.mult,
                    ALU.add)
                vnew.append(t2)
            return vnew


def _prep_inputs(inp, core):
    g = lambda k: np.asarray(inp[k], dtype=np.float32)
    xi, xt = g("xi"), g("xt")
    xiT = np.ascontiguousarray(xi.transpose(2, 0, 1).reshape(D, BL))
    xtT = np.ascontiguousarray(xt.transpose(2, 0, 1).reshape(D, BL))
    c = core
    W0c = np.ascontiguousarray(g("W0")[:, c * S:(c + 1) * S])
    W1c = np.ascontiguousarray(g("W1")[:, c * S:(c + 1) * S])

    def padM(M):  # (S, S*R) -> (S, S*RP), col order s*RP+r
        Mr = M.reshape(S, R, S).transpose(0, 2, 1)
        Mp = np.zeros((S, S, RP), np.float32)
        Mp[:, :, :R] = Mr
        return np.ascontiguousarray(Mp.reshape(S, S * RP))

    def padMb(Mb):  # (S*R,) -> per-partition (128, S*RP//128)
        Mbr = Mb.reshape(R, S).T
        Mbp = np.zeros((S, RP), np.float32)
        Mbp[:, :R] = Mbr
        return np.ascontiguousarray(Mbp.reshape(S * RP // 128, 128).T)

    bcore = c // 4
    selm = np.zeros((BL, L), np.float32)
    selm[np.arange(L) + bcore * L, np.arange(L)] = 1.0

    wc = g("Wout")[c * S:(c + 1) * S, 0]
    eye = np.eye(128, dtype=np.float16)
    eyeW = np.ascontiguousarray(
        (wc.astype(np.float16)[:, None, None] * eye)
        .transpose(1, 0, 2).reshape(128, S * 128))

    return {
        "xiT": xiT, "xtT": xtT,
        "Wv": g("Wv"), "bvr": np.ascontiguousarray(g("bv").reshape(1, H)),
        "Wk": g("Wk"), "bkp": np.ascontiguousarray(g("bk").reshape(4, 128).T),
        "Wq": g("Wq"), "bqp": np.ascontiguousarray(g("bq").reshape(4, 128).T),
        "Wm": g("Wm").astype(np.float16), "bmp": np.ascontiguousarray(g("bm").reshape(2, 128).T),
        "W0c": W0c, "b0c": np.ascontiguousarray(
            g("b0")[c * S:(c + 1) * S].reshape(S, 1)),
        "W1c": W1c, "b1c": np.ascontiguousarray(
            g("b1")[c * S:(c + 1) * S].reshape(S, 1)),
        "M0p": padM(g("M0")[c]), "Mb0p": padMb(g("Mb0")[c]),
        "M1p": padM(g("M1")[c]), "Mb1p": padMb(g("Mb1")[c]),
        "eyeW": eyeW,
        "eye16": np.eye(128, dtype=np.float16),
        "eye32": np.eye(128, dtype=np.float32),
        "ones1": np.ones((1, 128), np.float32),
        "ones128": np.ones((128, 1), np.float32),
        "sel": selm,
        "gam": np.ascontiguousarray(g("gamma").reshape(1, D)),
        "bet": np.ascontiguousarray(g("beta").reshape(1, D)),
    }


def kernel(**inputs):
    if "nc" not in _cache:
        _cache["nc"] = build_bass()
    nc = _cache["nc"]
    in_maps = [_prep_inputs(inputs, c) for c in range(NCORES)]
    res = run_bass_kernel_spmd(nc, in_maps, core_ids=list(range(NCORES)))
    vT = np.asarray(res.results[0]["vT_out"], dtype=np.float32)
    out = vT.reshape(D, B, L).transpose(1, 2, 0)
    return np.ascontiguousarray(out)


# revision 41
# speedup vs baseline: 1.1610x; 1.1610x over previous
"""Trainium2 Bass kernel for the BAF bilinear-attention-fusion module.

Sharding: one bilinear chunk c per NeuronCore (C=8 chunks, 8 cores).
Per layer each core computes its chunk's (B, Lq, Lk) score contribution
numer_c * rsqrt(norm_c); a ReduceScatter (f16) sums chunks and hands each
core a 64-query slice for softmax+attend; an AllGather rebuilds the
attended tensor for the (replicated) scramble + output projection + BN.

z post-processing uses the signed-sqrt identity p = sign(z)*sqrt(|z|)
computed as (z16 & 0x8000) | sqrt(|z16|): one Sqrt LUT pass on the Scalar
engine plus int16-bitcast DVE ops that hit the 4x all-SBUF fp16 perf mode.
The chunk norm rhs is |z16| itself (== p^2). PSUM-exit copies alternate
DVE/ACT to balance load. PE accumulates numer (w_s-scaled identities) and
norm (identity) per s-slice, software-pipelined two waves behind the z
matmuls. Key-side projections (xi -> keyT -> x0T -> a-factors) are
layer-invariant and computed once.
"""
import sys
sys.path.insert(0, '/opt/trn_rl_repo')

import numpy as np
import ml_dtypes  # noqa: F401

import concourse.bass as bass  # noqa: F401
import concourse.tile as tile
from concourse import bacc, mybir
from concourse.bass_utils import run_bass_kernel_spmd

# Restrict the activation-table universe to the two sets this kernel needs
# (sqrt for the z chain + BN, exp for softmax); the greedy act-table-load
# pass then emits the minimal ~5 table switches instead of ping-ponging.
from concourse import bacc as _bacc_mod
_orig_gat = _bacc_mod.get_activation_tables
_KEEP_TABLES = ("sqrt_and_others", "natural_log_exp_and_others")


def _forced_tables(arch):
    t = _orig_gat(arch)
    return {name: (fs if name in _KEEP_TABLES else set())
            for name, fs in t.items()}


_bacc_mod.get_activation_tables = _forced_tables

dt = mybir.dt
AF = mybir.ActivationFunctionType
ALU = mybir.AluOpType
f32 = dt.float32
f32r = dt.float32r
f16 = dt.float16
i16 = dt.int16

B, L, D, H, C, R = 2, 256, 256, 512, 8, 5
S = H // C            # 64
RP = 8                # rank padded to 8 (z-matmul K dim)
BL = B * L            # 512
NCORES = 8

_cache = {}


def build_bass():
    nc = bacc.Bacc("TRN2", target_bir_lowering=False, debug=False,
                   num_devices=NCORES)

    def register_const(value, dtype=f32):
        t = nc.alloc_sbuf_tensor(f"const-{dtype.name}-{value}", [128, 1], dtype)
        nc.gpsimd.memset(t.ap(), value)
        nc.const_aps.aps[(dtype, value)] = t.ap()

    register_const(1e-12)
    register_const(1e-5)
    nc.all_engine_barrier()

    def din(name, shape, dtype=f32r):
        return nc.dram_tensor(name, shape, dtype, kind="ExternalInput")

    dd = {}
    dd["xiT"] = din("xiT", [D, BL])
    dd["xtT"] = din("xtT", [D, BL])
    dd["Wv"] = din("Wv", [D, H]); dd["bvr"] = din("bvr", [1, H])
    dd["Wk"] = din("Wk", [D, H]); dd["bkp"] = din("bkp", [128, 4], f32)
    dd["Wq"] = din("Wq", [D, H]); dd["bqp"] = din("bqp", [128, 4], f32)
    dd["Wm"] = din("Wm", [H, D], f16); dd["bmp"] = din("bmp", [128, 2], f32)
    dd["W0c"] = din("W0c", [H, S]); dd["b0c"] = din("b0c", [S, 1], f32)
    dd["W1c"] = din("W1c", [H, S]); dd["b1c"] = din("b1c", [S, 1], f32)
    dd["M0p"] = din("M0p", [S, S * RP]); dd["Mb0p"] = din("Mb0p", [128, 4], f32)
    dd["M1p"] = din("M1p", [S, S * RP]); dd["Mb1p"] = din("Mb1p", [128, 4], f32)
    dd["eyeW"] = din("eyeW", [128, S * 128], f16)
    dd["eye16"] = din("eye16", [128, 128], f16)
    dd["eye32"] = din("eye32", [128, 128], f32)
    dd["ones1"] = din("ones1", [1, 128])
    dd["ones128"] = din("ones128", [128, 1])
    dd["sel"] = din("sel", [BL, L])
    dd["gam"] = din("gam", [1, D], f32)
    dd["bet"] = din("bet", [1, D], f32)
    dd["out"] = nc.dram_tensor("vT_out", [D, BL], f32r, kind="ExternalOutput")

    with tile.TileContext(nc) as tc:
        _program(nc, tc, dd)
    nc.finalize()
    return nc


def _program(nc, tc, dd):
    with tc.tile_pool(name="const", bufs=1) as cp, \
         tc.tile_pool(name="dram", bufs=1, space="DRAM") as dram:

        def load(dram_t, shape, dtype=f32r, tag=None):
            t = cp.tile(shape, dtype, tag=tag or dram_t.name)
            nc.sync.dma_start(t[:], dram_t[:])
            return t

        def load4(dram_t, rows, cols, n, dtype=f32r):
            return [load(dram_t[kt * 128:(kt + 1) * 128, :], [rows, cols],
                         dtype, tag=f"{dram_t.name}{kt}") for kt in range(n)]

        # load order = consumption order: key-side precompute deps first,
        # then query-side layer-1 deps, z-phase tables, then output-phase.
        xiT = load4(dd["xiT"], 128, BL, 2)
        Wk = load4(dd["Wk"], 128, H, 2)
        bkp = load(dd["bkp"], [128, 4], f32)
        W0c = load4(dd["W0c"], 128, S, 4)
        b0c = load(dd["b0c"], [S, 1], f32)
        M0p = load(dd["M0p"], [S, S * RP])
        Mb0p = load(dd["Mb0p"], [128, 4], f32)
        xtT = load4(dd["xtT"], 128, BL, 2)
        Wq = load4(dd["Wq"], 128, H, 2)
        bqp = load(dd["bqp"], [128, 4], f32)
        W1c = load4(dd["W1c"], 128, S, 4)
        b1c = load(dd["b1c"], [S, 1], f32)
        M1p = load(dd["M1p"], [S, S * RP])
        Mb1p = load(dd["Mb1p"], [128, 4], f32)
        eye16 = load(dd["eye16"], [128, 128], f16)
        eyeW = load(dd["eyeW"], [128, S * 128], f16)
        Wv = load4(dd["Wv"], 128, H, 2)
        bvr = load(dd["bvr"], [1, H])
        sel = load4(dd["sel"], 128, L, 4)
        eye32 = load(dd["eye32"], [128, 128], f32)
        ones1 = load(dd["ones1"], [1, 128])
        ones128 = load(dd["ones128"], [128, 1])
        Wm = load4(dd["Wm"], 128, D, 4, f16)
        bmp = load(dd["bmp"], [128, 2], f32)
        gam = load(dd["gam"], [1, D], f32)
        bet = load(dd["bet"], [1, D], f32)

        w = dict(cp=cp, xiT=xiT, Wv=Wv, Wq=Wq, Wm=Wm, W1c=W1c,
                 sel=sel, M1p=M1p, eyeW=eyeW, eye16=eye16,
                 eye32=eye32, ones1=ones1, ones128=ones128, bvr=bvr,
                 bqp=bqp, bmp=bmp, b1c=b1c, Mb1p=Mb1p, gam=gam, bet=bet)

        # ---- key-side precompute (layer-invariant: depends only on xi) ----
        aF = dram.tile([S * RP, BL], f16, tag="aF")
        with tc.tile_pool(name="kps", bufs=2, space="PSUM") as kps, \
             tc.tile_pool(name="ktmp", bufs=1) as ktmp:
            keyT = []
            for mt in range(4):
                p = kps.tile([128, BL], f32, tag="kproj")
                for kt in range(2):
                    nc.tensor.matmul(p[:], Wk[kt][:, mt * 128:(mt + 1) * 128],
                                     xiT[kt][:], start=(kt == 0), stop=(kt == 1))
                t = ktmp.tile([128, BL], f32r, tag=f"keyT{mt}")
                nc.scalar.activation(t[:], p[:], AF.Identity,
                                     bias=bkp[:, mt:mt + 1])
                keyT.append(t)
            p = kps.tile([S, BL], f32, tag="kprojS")
            for kt in range(4):
                nc.tensor.matmul(p[:], W0c[kt][:], keyT[kt][:],
                                 start=(kt == 0), stop=(kt == 3))
            x0T = ktmp.tile([S, BL], f32r, tag="x0T")
            nc.scalar.activation(x0T[:], p[:], AF.Identity, bias=b0c[:])
            for mt in range(4):
                p = kps.tile([128, BL], f32, tag="kproj")
                nc.tensor.matmul(p[:], M0p[:, mt * 128:(mt + 1) * 128],
                                 x0T[:], start=True, stop=True)
                t = ktmp.tile([128, BL], f16, tag=f"aT{mt}")
                nc.scalar.activation(t[:], p[:], AF.Identity,
                                     bias=Mb0p[:, mt:mt + 1])
                nc.sync.dma_start(aF[mt * 128:(mt + 1) * 128, :], t[:])
        w["aFv"] = aF[:].rearrange("(s r) bl -> r s bl", r=RP)

        vT = xtT
        for layer in range(2):
            vT = _layer(nc, tc, dram, layer, vT, w)

        for kt in range(2):
            nc.sync.dma_start(dd["out"][kt * 128:(kt + 1) * 128, :], vT[kt][:])


def _layer(nc, tc, dram, layer, vT, w):
    X = mybir.AxisListType.X
    RG = [list(range(NCORES))]
    L_ = layer
    bqp, b1c, Mb1p = w["bqp"], w["b1c"], w["Mb1p"]

    with tc.tile_pool(name=f"l{L_}sb", bufs=1) as sb:
        # ---------------- query-side projections ----------------
        bbF = dram.tile([S * RP, BL], f16, tag=f"bbF{L_}")
        with tc.tile_pool(name=f"l{L_}pps", bufs=2, space="PSUM") as ps:
            with tc.tile_pool(name=f"l{L_}ptmp", bufs=1) as ptmp:
                queryT = []
                for mt in range(4):
                    p = ps.tile([128, BL], f32, tag="proj")
                    for kt in range(2):
                        nc.tensor.matmul(p[:],
                                         w["Wq"][kt][:, mt * 128:(mt + 1) * 128],
                                         vT[kt][:],
                                         start=(kt == 0), stop=(kt == 1))
                    t = ptmp.tile([128, BL], f32r, tag=f"queryT{mt}")
                    nc.scalar.activation(t[:], p[:], AF.Identity,
                                         bias=bqp[:, mt:mt + 1])
                    queryT.append(t)

                p = ps.tile([S, BL], f32, tag="projS")
                for kt in range(4):
                    nc.tensor.matmul(p[:], w["W1c"][kt][:], queryT[kt][:],
                                     start=(kt == 0), stop=(kt == 3))
                x1T = sb.tile([S, BL], f32r, tag="x1T")
                nc.scalar.activation(x1T[:], p[:], AF.Identity, bias=b1c[:])

            for mt in range(4):
                p = ps.tile([128, BL], f32, tag="proj")
                nc.tensor.matmul(p[:], w["M1p"][:, mt * 128:(mt + 1) * 128],
                                 x1T[:], start=True, stop=True)
                t = sb.tile([128, BL], f16, tag=f"bT{mt}")
                nc.scalar.activation(t[:], p[:], AF.Identity,
                                     bias=Mb1p[:, mt:mt + 1])
                nc.sync.dma_start(bbF[mt * 128:(mt + 1) * 128, :], t[:])
        bbFv = bbF[:].rearrange("(s r) bl -> r s bl", r=RP)
        aFv = w["aFv"]

        # ---------------- bilinear z phase ----------------
        # one contrib slab + ReduceScatter per (b,ih) block: RS#k overlaps
        # the z compute of later blocks; core c owns score rows
        # {128k + 16c + t}, i.e. 16 rows of each block.
        contribs = dram.tile([BL, L], f16, tag="contribs")
        with tc.tile_pool(name=f"l{L_}zw", bufs=3, space="PSUM") as zp, \
             tc.tile_pool(name=f"l{L_}acc", bufs=1, space="PSUM") as accp, \
             tc.tile_pool(name=f"l{L_}zsb", bufs=5) as zsb, \
             tc.tile_pool(name=f"l{L_}zsb2", bufs=4) as zsb2, \
             tc.tile_pool(name=f"l{L_}zsbP", bufs=3) as zsbP, \
             tc.tile_pool(name=f"l{L_}fac", bufs=2) as fac:

            def emit_accums(item):
                wv, pw_t, a16_t, numer_t, norm_t = item
                for sl in range(4):
                    s = 4 * wv + sl
                    nc.tensor.matmul(numer_t[:],
                                     w["eyeW"][:, s * 128:(s + 1) * 128],
                                     pw_t[:, sl * L:(sl + 1) * L],
                                     start=(s == 0), stop=(s == 63))
                    nc.tensor.matmul(norm_t[:], w["eye16"][:],
                                     a16_t[:, sl * L:(sl + 1) * L],
                                     start=(s == 0), stop=(s == 63))

            def load_group(g, b, i0, store):
                ta = fac.tile([RP, 16 * L], f16, tag=f"awt{g % 2}")
                nc.sync.dma_start(
                    ta[:].rearrange("r (s j) -> r s j", s=16),
                    aFv[:, g * 16:(g + 1) * 16, b * L:(b + 1) * L])
                tb = fac.tile([RP, 16 * 128], f16, tag=f"bwt{g % 2}")
                nc.sync.dma_start(
                    tb[:].rearrange("r (s i) -> r s i", s=16),
                    bbFv[:, g * 16:(g + 1) * 16, i0:i0 + 128])
                store[g] = (ta, tb)

            for b in range(B):
                for ih in range(2):
                    i0 = b * L + ih * 128
                    numer = accp.tile([128, L], f32, tag="numer")
                    norm = accp.tile([128, L], f32, tag="norm")
                    grp = {}
                    load_group(0, b, i0, grp)
                    load_group(1, b, i0, grp)
                    # Software-pipelined emission: per wave, each engine's
                    # stream only touches data from earlier waves so no
                    # engine queue head-of-line blocks on the current wave.
                    # The f16 pairs are bitcast int32 (bitwise ops are
                    # DVE-only and 32-bit-only on this hw): abs masks both
                    # halves' sign bits; the sign-merge is one fused
                    # (z & 0x80008000) + sq -- fp16 magnitudes stay below
                    # 0x8000 so the add cannot carry.
                    zt = {}

                    def emit_sq(wv):
                        sq = zsb2.tile([128, 4 * L], f16, tag="sq")
                        nc.scalar.activation(sq[:], zt[wv][1][:], AF.Sqrt,
                                             bias=0.0)
                        zt[wv] = (zt[wv][0], zt[wv][1], sq)

                    def emit_pw(wv):
                        z16, a16, sq = zt[wv]
                        pw = zsbP.tile([128, 4 * L], f16, tag="pw")
                        nc.vector.scalar_tensor_tensor(
                            pw[:].bitcast(dt.int32), z16[:].bitcast(dt.int32),
                            0x80008000 - (1 << 32), sq[:].bitcast(dt.int32),
                            ALU.bitwise_and, ALU.add)
                        zt[wv] = (z16, a16, sq, pw)

                    def emit_acc(wv):
                        emit_accums((wv, zt[wv][3], zt[wv][1], numer, norm))
                        del zt[wv]

                    for wv in range(16):
                        g, part = wv // 4, wv % 4
                        if part == 0 and g + 2 < 4:
                            load_group(g + 2, b, i0, grp)
                        awt, bwt = grp[g]
                        zw = zp.tile([128, 4 * L], f32, tag="zw")
                        for sl in range(4):
                            k = part * 4 + sl
                            nc.tensor.matmul(
                                zw[:, sl * L:(sl + 1) * L],
                                bwt[:, k * 128:(k + 1) * 128],
                                awt[:, k * L:(k + 1) * L],
                                start=True, stop=True)
                        if wv >= 2:
                            emit_pw(wv - 2)
                        z16 = zsb2.tile([128, 4 * L], f16, tag="z16")
                        if wv % 16 in (1, 3, 5, 7, 9, 11, 13):
                            nc.vector.tensor_copy(z16[:], zw[:])
                        else:
                            nc.scalar.activation(z16[:], zw[:], AF.Identity,
                                                 bias=0.0)
                        a16 = zsb.tile([128, 4 * L], f16, tag="a16")
                        nc.vector.tensor_scalar(
                            a16[:].bitcast(dt.int32), z16[:].bitcast(dt.int32),
                            0x7FFF7FFF, None, ALU.bitwise_and)
                        zt[wv] = (z16, a16)
                        if wv >= 1:
                            emit_sq(wv - 1)
                        if wv >= 3:
                            emit_acc(wv - 3)
                    emit_sq(15)
                    emit_pw(14)
                    emit_pw(15)
                    for wv in (13, 14, 15):
                        emit_acc(wv)
                    sqn = zsb2.tile([128, L], f32, tag="sqn")
                    nc.scalar.activation(sqn[:], norm[:], AF.Sqrt, bias=1e-12)
                    rstd = zsb2.tile([128, L], f32, tag="rstd")
                    nc.vector.reciprocal(rstd[:], sqn[:])
                    contrib = zsb2.tile([128, L], f16, tag="contrib")
                    nc.vector.scalar_tensor_tensor(
                        contrib[:], numer[:], 1.0, rstd[:], ALU.mult, ALU.mult)
                    nc.sync.dma_start(contribs[i0:i0 + 128, :], contrib[:])

        # ---------- ReduceScatter scores (f16) ----------
        scores_d = dram.tile([S, L], f16, tag="scores")
        nc.gpsimd.collective_compute("ReduceScatter", ALU.add,
                                     ins=[contribs.opt()],
                                     outs=[scores_d.opt()],
                                     replica_groups=RG)

        # ---------- value projection (overlaps the RS chain) ----------
        # value tiles are copied out of PSUM directly in h-column e-order
        # (evens then odds) so they serve as the attend rhs per batch and
        # the scramble gather stays contiguous.
        atted_d = dram.tile([S, H], f16, tag="atted")
        with tc.tile_pool(name=f"l{L_}aps", bufs=2, space="PSUM") as aps:
            vsel = []
            value = []
            for mt in range(4):
                p = aps.tile([128, H], f32, tag="vproj")
                for kt in range(2):
                    nc.tensor.matmul(p[:], vT[kt][:, mt * 128:(mt + 1) * 128],
                                     w["Wv"][kt][:], start=(kt == 0), stop=False)
                nc.tensor.matmul(p[:], w["ones1"][:, 0:128], w["bvr"][:],
                                 start=False, stop=True)
                t = sb.tile([128, H], f32r, tag=f"value{mt}")
                nc.vector.tensor_copy(t[:], p[:])
                value.append(t)

            # value rows for this core's batch, h-columns permuted to
            # e-order (evens then odds) so the scramble gather is contiguous
            vsel = []
            for jt in range(2):
                p = aps.tile([128, H], f32, tag="vsel")
                for kt in range(4):
                    nc.tensor.matmul(p[:],
                                     w["sel"][kt][:, jt * 128:(jt + 1) * 128],
                                     value[kt][:],
                                     start=(kt == 0), stop=(kt == 3))
                t = sb.tile([128, H], f32r, tag=f"vsel{jt}")
                nc.vector.tensor_copy(
                    t[:].rearrange("j (ih iq) -> j ih iq", ih=2),
                    p[:].rearrange("j (iq ih) -> j ih iq", ih=2))
                vsel.append(t)

            # ---------- softmax + attend ----------
            # scores are bounded (|s| < ~30) so exp cannot overflow fp32:
            # skip the max-subtraction for a shorter serial chain.
            sc16 = sb.tile([S, L], f16, tag="sc16")
            nc.sync.dma_start(sc16[:], scores_d[:])
            # attend on unnormalized exp scores; the softmax denominator is
            # folded into the atted PSUM-exit as a per-row ACT scale.
            ex = sb.tile([S, L], f32, tag="ex")
            sume = sb.tile([S, 1], f32, tag="sume")
            nc.scalar.activation(ex[:], sc16[:], AF.Exp, bias=0.0,
                                 accum_out=sume[:])
            rcp = sb.tile([S, 1], f32, tag="rcp")
            nc.vector.reciprocal(rcp[:], sume[:])

            attT = []
            for jt in range(2):
                p = aps.tile([128, S], f32, tag="attT")
                nc.tensor.transpose(p[:], ex[:, jt * 128:(jt + 1) * 128],
                                    w["eye32"][0:S, 0:S])
                t = sb.tile([128, S], f32r, tag=f"attT{jt}")
                nc.vector.tensor_copy(t[:], p[:])
                attT.append(t)

            pa = aps.tile([S, H], f32, tag="atted")
            for jt in range(2):
                nc.tensor.matmul(pa[:], attT[jt][:], vsel[jt][:],
                                 start=(jt == 0), stop=(jt == 1))
            ats = sb.tile([S, H], f16, tag="atteds")
            nc.scalar.activation(ats[:], pa[:], AF.Identity, bias=0.0,
                                 scale=rcp[:])
            nc.sync.dma_start(atted_d[:], ats[:])

        # ---------- AllGather (f16 payload); scramble; Wm; BN ----------
        attedall = dram.tile([BL, H], f16, tag="attedall")
        nc.gpsimd.collective_compute("AllGather", ALU.bypass,
                                     ins=[atted_d.opt()],
                                     outs=[attedall.opt()],
                                     replica_groups=RG)

        with tc.tile_pool(name=f"l{L_}fps", bufs=1, space="PSUM") as fps:
            # scrT[j', (b,i')] = atted_perm[b*256 + j'%256, (j'//256)*256 + i']
            aa = attedall[:].rearrange("(b lh p) (ih iq) -> lh ih p b iq",
                                       b=2, lh=2, ih=2)
            scrT = []
            for jt in range(4):
                t = sb.tile([128, BL], f16, tag=f"scrT{jt}")
                nc.sync.dma_start(
                    t[:].rearrange("p (b iq) -> p b iq", b=2),
                    aa[jt % 2, jt // 2])
                scrT.append(t)

            attnT = []
            for mt in range(2):
                p = fps.tile([128, BL], f32, tag="attnT")
                for kt in range(4):
                    nc.tensor.matmul(p[:],
                                     w["Wm"][kt][:, mt * 128:(mt + 1) * 128],
                                     scrT[kt][:],
                                     start=(kt == 0), stop=(kt == 3))
                t = sb.tile([128, BL], f32r, tag=f"attnT{mt}")
                nc.scalar.activation(t[:], p[:], AF.Identity,
                                     bias=w["bmp"][:, mt:mt + 1])
                attnT.append(t)

            cs = fps.tile([1, BL], f32, tag="cs")
            for mt in range(2):
                nc.tensor.matmul(cs[:], w["ones128"][:], attnT[mt][:],
                                 start=(mt == 0), stop=(mt == 1))
            cs2 = fps.tile([1, BL], f32, tag="cs2")
            sqs = []
            for mt in range(2):
                sq = sb.tile([128, BL], f32r, tag=f"sq{mt}")
                nc.scalar.activation(sq[:], attnT[mt][:].bitcast(f32),
                                     AF.Square)
                sqs.append(sq)
            for mt in range(2):
                nc.tensor.matmul(cs2[:], w["ones128"][:], sqs[mt][:],
                                 start=(mt == 0), stop=(mt == 1))

            def stt(out, in0, scalar, in1, op0, op1):
                nc.vector.scalar_tensor_tensor(out, in0, scalar, in1, op0, op1)

            # per-position sums read straight from PSUM; 1/BL folds into
            # the mu scale and the Sqrt input scale.
            csum = sb.tile([1, D], f32, tag="csum")
            nc.vector.tensor_tensor(csum[:], cs[:, 0:D], cs[:, D:BL], ALU.add)
            c2sum = sb.tile([1, D], f32, tag="c2sum")
            nc.vector.tensor_tensor(c2sum[:], cs2[:, 0:D], cs2[:, D:BL],
                                    ALU.add)
            mu = sb.tile([1, D], f32, tag="mu")
            nc.vector.tensor_scalar_mul(mu[:], csum[:], 1.0 / BL)
            varBL = sb.tile([1, D], f32, tag="varBL")
            stt(varBL[:], mu[:], -1.0, csum[:], ALU.mult, ALU.mult)
            stt(varBL[:], varBL[:], 1.0, c2sum[:], ALU.mult, ALU.add)
            sqv = sb.tile([1, D], f32, tag="sqv")
            nc.scalar.activation(sqv[:], varBL[:], AF.Sqrt, bias=1e-5,
                                 scale=1.0 / BL)
            rstd = sb.tile([1, D], f32, tag="rstdb")
            nc.vector.reciprocal(rstd[:], sqv[:])
            Am = sb.tile([1, D], f32, tag="Am")
            stt(Am[:], w["gam"][:], 1.0, rstd[:], ALU.mult, ALU.mult)
            Bb = sb.tile([1, D], f32, tag="Bb")
            stt(Bb[:], mu[:], -1.0, Am[:], ALU.mult, ALU.mult)
            stt(Bb[:], Bb[:], 1.0, w["bet"][:], ALU.mult, ALU.add)
            A2 = sb.tile([1, BL], f32r, tag="A2")
            nc.vector.tensor_copy(A2[:, 0:D], Am[:])
            nc.vector.tensor_copy(A2[:, D:BL], Am[:])
            B2 = sb.tile([1, BL], f32r, tag="B2")
            nc.vector.tensor_copy(B2[:, 0:D], Bb[:])
            nc.vector.tensor_copy(B2[:, D:BL], Bb[:])
            Abc = fps.tile([128, BL], f32, tag="Abc")
            nc.tensor.matmul(Abc[:], w["ones1"][:, 0:128], A2[:],
                             start=True, stop=True)
            Bbc = fps.tile([128, BL], f32, tag="Bbc")
            nc.tensor.matmul(Bbc[:], w["ones1"][:, 0:128], B2[:],
                             start=True, stop=True)

            vnew = []
            for mt in range(2):
                t1 = sb.tile([128, BL], f32, tag="t1")
                stt(t1[:], attnT[mt][:].bitcast(f32), 1.0, Abc[:],
                    ALU.mult, ALU.mult)
                stt(t1[:], t1[:], 1.0, Bbc[:], ALU.mult, ALU.add)
                nc.vector.tensor_scalar_max(t1[:], t1[:], 0.0)
                t2 = w["cp"].tile([128, BL], f32r, tag=f"vnew{L_}{mt}")
                stt(t2[:], t1[:], 1.0, vT[mt][:].bitcast(f32), ALU.mult,
                    ALU.add)
                vnew.append(t2)
            return vnew


def _prep_inputs(inp, core):
    g = lambda k: np.asarray(inp[k], dtype=np.float32)
    xi, xt = g("xi"), g("xt")
    xiT = np.ascontiguousarray(xi.transpose(2, 0, 1).reshape(D, BL))
    xtT = np.ascontiguousarray(xt.transpose(2, 0, 1).reshape(D, BL))
    c = core
    W0c = np.ascontiguousarray(g("W0")[:, c * S:(c + 1) * S])
    W1c = np.ascontiguousarray(g("W1")[:, c * S:(c + 1) * S])

    def padM(M):  # (S, S*R) -> (S, S*RP), col order s*RP+r
        Mr = M.reshape(S, R, S).transpose(0, 2, 1)
        Mp = np.zeros((S, S, RP), np.float32)
        Mp[:, :, :R] = Mr
        return np.ascontiguousarray(Mp.reshape(S, S * RP))

    def padMb(Mb):  # (S*R,) -> per-partition (128, S*RP//128)
        Mbr = Mb.reshape(R, S).T
        Mbp = np.zeros((S, RP), np.float32)
        Mbp[:, :R] = Mbr
        return np.ascontiguousarray(Mbp.reshape(S * RP // 128, 128).T)

    bcore = c // 4
    selm = np.zeros((BL, L), np.float32)
    selm[np.arange(L) + bcore * L, np.arange(L)] = 1.0

    wc = g("Wout")[c * S:(c + 1) * S, 0]
    eye = np.eye(128, dtype=np.float16)
    eyeW = np.ascontiguousarray(
        (wc.astype(np.float16)[:, None, None] * eye)
        .transpose(1, 0, 2).reshape(128, S * 128))

    return {
        "xiT": xiT, "xtT": xtT,
        "Wv": g("Wv"), "bvr": np.ascontiguousarray(g("bv").reshape(1, H)),
        "Wk": g("Wk"), "bkp": np.ascontiguousarray(g("bk").reshape(4, 128).T),
        "Wq": g("Wq"), "bqp": np.ascontiguousarray(g("bq").reshape(4, 128).T),
        "Wm": g("Wm").astype(np.float16), "bmp": np.ascontiguousarray(g("bm").reshape(2, 128).T),
        "W0c": W0c, "b0c": np.ascontiguousarray(
            g("b0")[c * S:(c + 1) * S].reshape(S, 1)),
        "W1c": W1c, "b1c": np.ascontiguousarray(
            g("b1")[c * S:(c + 1) * S].reshape(S, 1)),
        "M0p": padM(g("M0")[c]), "Mb0p": padMb(g("Mb0")[c]),
        "M1p": padM(g("M1")[c]), "Mb1p": padMb(g("Mb1")[c]),
        "eyeW": eyeW,
        "eye16": np.eye(128, dtype=np.float16),
        "eye32": np.eye(128, dtype=np.float32),
        "ones1": np.ones((1, 128), np.float32),
        "ones128": np.ones((128, 1), np.float32),
        "sel": selm,
        "gam": np.ascontiguousarray(g("gamma").reshape(1, D)),
        "bet": np.ascontiguousarray(g("beta").reshape(1, D)),
    }


def kernel(**inputs):
    if "nc" not in _cache:
        _cache["nc"] = build_bass()
    nc = _cache["nc"]
    in_maps = [_prep_inputs(inputs, c) for c in range(NCORES)]
    res = run_bass_kernel_spmd(nc, in_maps, core_ids=list(range(NCORES)))
    vT = np.asarray(res.results[0]["vT_out"], dtype=np.float32)
    out = vT.reshape(D, B, L).transpose(1, 2, 0)
    return np.ascontiguousarray(out)


# revision 62
# speedup vs baseline: 1.2475x; 1.0745x over previous
"""Trainium2 Bass kernel for the BAF bilinear-attention-fusion module.

Sharding: one bilinear chunk c per NeuronCore (C=8 chunks, 8 cores).
Per layer each core computes its chunk's (B, Lq, Lk) score contribution
numer_c * rsqrt(norm_c); a ReduceScatter (f16) sums chunks and hands each
core a 64-query slice for softmax+attend; an AllGather (f16) rebuilds the
attended tensor for the (replicated) scramble + output projection + BN.

z post-processing uses the signed-sqrt identity p = sign(z)*sqrt(|z|):
one Sqrt LUT pass on the Scalar engine, abs/sign-extract as int32-pair
bitmasks on DVE (bitwise ops are DVE-only and 32-bit-only on this hw;
f16 pairs are bitcast int32 and both halves masked at once), and the
sign|sqrt merge as an int add on the otherwise-idle Pool engine (fp16
magnitudes < 0x8000 so the add cannot carry). PSUM-exit copies split
ACT/DVE by a tuned wave pattern. PE accumulates numer (w_s-scaled
identities) and norm (plain identity) per s-slice; all per-wave stages
are emitted software-pipelined (sq one wave behind, the sign-merge two,
accumulation four) so no engine queue head-of-line blocks on the
current wave. Key-side projections (xi -> keyT -> x0T -> a-factors) are
layer-invariant and computed once; value/attend work is emitted after
the ReduceScatter issue to overlap the collective; BatchNorm uses
Sqrt+reciprocal (Rsqrt LUT is blocked) with 1/BL folded into the ACT
input scale. Collectives occupy the Pool track in the cost model, so
the first z block of layer 1 (which overlaps layer 0's AllGather) does
its sign-merge on DVE instead of Pool.
"""
import sys
sys.path.insert(0, '/opt/trn_rl_repo')

import numpy as np
import ml_dtypes  # noqa: F401

import concourse.bass as bass  # noqa: F401
import concourse.tile as tile
from concourse import bacc, mybir
from concourse.bass_utils import run_bass_kernel_spmd

# Restrict the activation-table universe to the two sets this kernel needs
# (sqrt for the z chain + BN, exp for softmax); the greedy act-table-load
# pass then emits the minimal ~5 table switches instead of ping-ponging.
from concourse import bacc as _bacc_mod
_orig_gat = _bacc_mod.get_activation_tables
_KEEP_TABLES = ("sqrt_and_others", "natural_log_exp_and_others")


def _forced_tables(arch):
    t = _orig_gat(arch)
    return {name: (fs if name in _KEEP_TABLES else set())
            for name, fs in t.items()}


_bacc_mod.get_activation_tables = _forced_tables

dt = mybir.dt
AF = mybir.ActivationFunctionType
ALU = mybir.AluOpType
f32 = dt.float32
f32r = dt.float32r
f16 = dt.float16
i16 = dt.int16

B, L, D, H, C, R = 2, 256, 256, 512, 8, 5
S = H // C            # 64
RP = 8                # rank padded to 8 (z-matmul K dim)
BL = B * L            # 512
NCORES = 8

_cache = {}


def build_bass():
    nc = bacc.Bacc("TRN2", target_bir_lowering=False, debug=False,
                   num_devices=NCORES)

    def register_const(value, dtype=f32):
        t = nc.alloc_sbuf_tensor(f"const-{dtype.name}-{value}", [128, 1], dtype)
        nc.gpsimd.memset(t.ap(), value)
        nc.const_aps.aps[(dtype, value)] = t.ap()

    register_const(1e-12)
    register_const(1e-5)
    nc.all_engine_barrier()

    def din(name, shape, dtype=f32r):
        return nc.dram_tensor(name, shape, dtype, kind="ExternalInput")

    dd = {}
    dd["xiT"] = din("xiT", [D, BL])
    dd["xtT"] = din("xtT", [D, BL])
    dd["Wv"] = din("Wv", [D, H]); dd["bvr"] = din("bvr", [1, H])
    dd["Wk"] = din("Wk", [D, H]); dd["bkp"] = din("bkp", [128, 4], f32)
    dd["Wq"] = din("Wq", [D, H]); dd["bqp"] = din("bqp", [128, 4], f32)
    dd["Wm"] = din("Wm", [H, D], f16); dd["bmp"] = din("bmp", [128, 2], f32)
    dd["W0c"] = din("W0c", [H, S]); dd["b0c"] = din("b0c", [S, 1], f32)
    dd["W1c"] = din("W1c", [H, S]); dd["b1c"] = din("b1c", [S, 1], f32)
    dd["M0p"] = din("M0p", [S, S * RP]); dd["Mb0p"] = din("Mb0p", [128, 4], f32)
    dd["M1p"] = din("M1p", [S, S * RP]); dd["Mb1p"] = din("Mb1p", [128, 4], f32)
    dd["eyeW"] = din("eyeW", [128, S * 128], f16)
    dd["eye16"] = din("eye16", [128, 128], f16)
    dd["eye32"] = din("eye32", [128, 128], f32)
    dd["ones1"] = din("ones1", [1, 128])
    dd["ones128"] = din("ones128", [128, 1])
    dd["sel"] = din("sel", [BL, L])
    dd["gam"] = din("gam", [1, D], f32)
    dd["bet"] = din("bet", [1, D], f32)
    dd["out"] = nc.dram_tensor("vT_out", [D, BL], f32r, kind="ExternalOutput")

    with tile.TileContext(nc) as tc:
        _program(nc, tc, dd)
    nc.finalize()
    return nc


def _program(nc, tc, dd):
    with tc.tile_pool(name="const", bufs=1) as cp, \
         tc.tile_pool(name="dram", bufs=1, space="DRAM") as dram:

        _lq = [0]

        def load(dram_t, shape, dtype=f32r, tag=None):
            t = cp.tile(shape, dtype, tag=tag or dram_t.name)
            eng = nc.sync if _lq[0] % 2 == 0 else nc.scalar
            _lq[0] += 1
            eng.dma_start(t[:], dram_t[:])
            return t

        def load4(dram_t, rows, cols, n, dtype=f32r):
            return [load(dram_t[kt * 128:(kt + 1) * 128, :], [rows, cols],
                         dtype, tag=f"{dram_t.name}{kt}") for kt in range(n)]

        # load order = consumption order: key-side precompute deps first,
        # then query-side layer-1 deps, z-phase tables, then output-phase.
        xiT = load4(dd["xiT"], 128, BL, 2)
        Wk = load4(dd["Wk"], 128, H, 2)
        bkp = load(dd["bkp"], [128, 4], f32)
        W0c = load4(dd["W0c"], 128, S, 4)
        b0c = load(dd["b0c"], [S, 1], f32)
        M0p = load(dd["M0p"], [S, S * RP])
        Mb0p = load(dd["Mb0p"], [128, 4], f32)
        xtT = load4(dd["xtT"], 128, BL, 2)
        Wq = load4(dd["Wq"], 128, H, 2)
        bqp = load(dd["bqp"], [128, 4], f32)
        W1c = load4(dd["W1c"], 128, S, 4)
        b1c = load(dd["b1c"], [S, 1], f32)
        M1p = load(dd["M1p"], [S, S * RP])
        Mb1p = load(dd["Mb1p"], [128, 4], f32)
        eye16 = load(dd["eye16"], [128, 128], f16)
        eyeW = load(dd["eyeW"], [128, S * 128], f16)
        Wv = load4(dd["Wv"], 128, H, 2)
        bvr = load(dd["bvr"], [1, H])
        sel = load4(dd["sel"], 128, L, 4)
        eye32 = load(dd["eye32"], [128, 128], f32)
        ones1 = load(dd["ones1"], [1, 128])
        ones128 = load(dd["ones128"], [128, 1])
        Wm = load4(dd["Wm"], 128, D, 4, f16)
        bmp = load(dd["bmp"], [128, 2], f32)
        gam = load(dd["gam"], [1, D], f32)
        bet = load(dd["bet"], [1, D], f32)

        w = dict(cp=cp, xiT=xiT, Wv=Wv, Wq=Wq, Wm=Wm, W1c=W1c,
                 sel=sel, M1p=M1p, eyeW=eyeW, eye16=eye16,
                 eye32=eye32, ones1=ones1, ones128=ones128, bvr=bvr,
                 bqp=bqp, bmp=bmp, b1c=b1c, Mb1p=Mb1p, gam=gam, bet=bet)

        # ---- key-side precompute (layer-invariant: depends only on xi) ----
        aF = dram.tile([S * RP, BL], f16, tag="aF")
        with tc.tile_pool(name="kps", bufs=2, space="PSUM") as kps, \
             tc.tile_pool(name="ktmp", bufs=1) as ktmp:
            keyT = []
            for mt in range(4):
                p = kps.tile([128, BL], f32, tag="kproj")
                for kt in range(2):
                    nc.tensor.matmul(p[:], Wk[kt][:, mt * 128:(mt + 1) * 128],
                                     xiT[kt][:], start=(kt == 0), stop=(kt == 1))
                t = ktmp.tile([128, BL], f32r, tag=f"keyT{mt}")
                nc.scalar.activation(t[:], p[:], AF.Identity,
                                     bias=bkp[:, mt:mt + 1])
                keyT.append(t)
            p = kps.tile([S, BL], f32, tag="kprojS")
            for kt in range(4):
                nc.tensor.matmul(p[:], W0c[kt][:], keyT[kt][:],
                                 start=(kt == 0), stop=(kt == 3))
            x0T = ktmp.tile([S, BL], f32r, tag="x0T")
            nc.scalar.activation(x0T[:], p[:], AF.Identity, bias=b0c[:])
            for mt in range(4):
                p = kps.tile([128, BL], f32, tag="kproj")
                nc.tensor.matmul(p[:], M0p[:, mt * 128:(mt + 1) * 128],
                                 x0T[:], start=True, stop=True)
                t = ktmp.tile([128, BL], f16, tag=f"aT{mt}")
                nc.scalar.activation(t[:], p[:], AF.Identity,
                                     bias=Mb0p[:, mt:mt + 1])
                nc.sync.dma_start(aF[mt * 128:(mt + 1) * 128, :], t[:])
        w["aFv"] = aF[:].rearrange("(s r) bl -> r s bl", r=RP)

        vT = xtT
        for layer in range(2):
            vT = _layer(nc, tc, dram, layer, vT, w)

        for kt in range(2):
            nc.sync.dma_start(dd["out"][kt * 128:(kt + 1) * 128, :], vT[kt][:])


def _layer(nc, tc, dram, layer, vT, w):
    X = mybir.AxisListType.X
    RG = [list(range(NCORES))]
    L_ = layer
    bqp, b1c, Mb1p = w["bqp"], w["b1c"], w["Mb1p"]

    with tc.tile_pool(name=f"l{L_}sb", bufs=1) as sb:
        # ---------------- query-side projections ----------------
        bbF = dram.tile([S * RP, BL], f16, tag=f"bbF{L_}")
        with tc.tile_pool(name=f"l{L_}pps", bufs=2, space="PSUM") as ps:
            with tc.tile_pool(name=f"l{L_}ptmp", bufs=1) as ptmp:
                queryT = []
                for mt in range(4):
                    p = ps.tile([128, BL], f32, tag="proj")
                    for kt in range(2):
                        nc.tensor.matmul(p[:],
                                         w["Wq"][kt][:, mt * 128:(mt + 1) * 128],
                                         vT[kt][:],
                                         start=(kt == 0), stop=(kt == 1))
                    t = ptmp.tile([128, BL], f32r, tag=f"queryT{mt}")
                    nc.scalar.activation(t[:], p[:], AF.Identity,
                                         bias=bqp[:, mt:mt + 1])
                    queryT.append(t)

                p = ps.tile([S, BL], f32, tag="projS")
                for kt in range(4):
                    nc.tensor.matmul(p[:], w["W1c"][kt][:], queryT[kt][:],
                                     start=(kt == 0), stop=(kt == 3))
                x1T = sb.tile([S, BL], f32r, tag="x1T")
                nc.scalar.activation(x1T[:], p[:], AF.Identity, bias=b1c[:])

            for mt in range(4):
                p = ps.tile([128, BL], f32, tag="proj")
                nc.tensor.matmul(p[:], w["M1p"][:, mt * 128:(mt + 1) * 128],
                                 x1T[:], start=True, stop=True)
                t = sb.tile([128, BL], f16, tag=f"bT{mt}")
                nc.scalar.activation(t[:], p[:], AF.Identity,
                                     bias=Mb1p[:, mt:mt + 1])
                nc.sync.dma_start(bbF[mt * 128:(mt + 1) * 128, :], t[:])
        bbFv = bbF[:].rearrange("(s r) bl -> r s bl", r=RP)
        aFv = w["aFv"]

        # ---------------- bilinear z phase ----------------
        # one contrib slab + ReduceScatter per (b,ih) block: RS#k overlaps
        # the z compute of later blocks; core c owns score rows
        # {128k + 16c + t}, i.e. 16 rows of each block.
        contribs = dram.tile([BL, L], f16, tag="contribs")
        with tc.tile_pool(name=f"l{L_}zw", bufs=3, space="PSUM") as zp, \
             tc.tile_pool(name=f"l{L_}acc", bufs=1, space="PSUM") as accp, \
             tc.tile_pool(name=f"l{L_}zsb", bufs=6) as zsb, \
             tc.tile_pool(name=f"l{L_}zsb2", bufs=4) as zsb2, \
             tc.tile_pool(name=f"l{L_}zsbS", bufs=3) as zsbS, \
             tc.tile_pool(name=f"l{L_}zsbG", bufs=2) as zsbG, \
             tc.tile_pool(name=f"l{L_}zsbT", bufs=2) as zsbT, \
             tc.tile_pool(name=f"l{L_}zsbP", bufs=3) as zsbP, \
             tc.tile_pool(name=f"l{L_}fac", bufs=2) as fac:

            def emit_accums(item):
                wv, pw_t, a16_t, numer_t, norm_t = item
                for sl in range(4):
                    s = 4 * wv + sl
                    nc.tensor.matmul(numer_t[:],
                                     w["eyeW"][:, s * 128:(s + 1) * 128],
                                     pw_t[:, sl * L:(sl + 1) * L],
                                     start=(s == 0), stop=(s == 63))
                    nc.tensor.matmul(norm_t[:], w["eye16"][:],
                                     a16_t[:, sl * L:(sl + 1) * L],
                                     start=(s == 0), stop=(s == 63))

            def load_group(g, b, i0, store):
                ta = fac.tile([RP, 16 * L], f16, tag=f"awt{g % 2}")
                nc.sync.dma_start(
                    ta[:].rearrange("r (s j) -> r s j", s=16),
                    aFv[:, g * 16:(g + 1) * 16, b * L:(b + 1) * L])
                tb = fac.tile([RP, 16 * 128], f16, tag=f"bwt{g % 2}")
                nc.sync.dma_start(
                    tb[:].rearrange("r (s i) -> r s i", s=16),
                    bbFv[:, g * 16:(g + 1) * 16, i0:i0 + 128])
                store[g] = (ta, tb)

            for b in range(B):
                for ih in range(2):
                    i0 = b * L + ih * 128
                    shield = False
                    numer = accp.tile([128, L], f32, tag="numer")
                    norm = accp.tile([128, L], f32, tag="norm")
                    grp = {}
                    load_group(0, b, i0, grp)
                    load_group(1, b, i0, grp)
                    # Software-pipelined emission: per wave, each engine's
                    # stream only touches data from earlier waves so no
                    # engine queue head-of-line blocks on the current wave.
                    # The f16 pairs are bitcast int32 (bitwise ops are
                    # DVE-only and 32-bit-only on this hw): abs masks both
                    # halves' sign bits; the sign-merge is one fused
                    # (z & 0x80008000) + sq -- fp16 magnitudes stay below
                    # 0x8000 so the add cannot carry.
                    zt = {}

                    def emit_sq(wv):
                        sq = zsbS.tile([128, 4 * L], f16, tag="sq")
                        nc.scalar.activation(sq[:], zt[wv][1][:], AF.Sqrt,
                                             bias=0.0)
                        zt[wv] = (zt[wv][0], zt[wv][1], sq)

                    def emit_pw(wv):
                        z16, a16, sq = zt[wv]
                        sgn = zsbG.tile([128, 4 * L], f16, tag="sgn")
                        nc.vector.tensor_scalar(
                            sgn[:].bitcast(dt.int32), z16[:].bitcast(dt.int32),
                            0x80008000 - (1 << 32), None, ALU.bitwise_and)
                        pw = zsbP.tile([128, 4 * L], f16, tag="pw")
                        if shield:
                            # AllGather in flight occupies Pool: merge on DVE
                            nc.vector.tensor_tensor(
                                pw[:].bitcast(dt.int32),
                                sgn[:].bitcast(dt.int32),
                                sq[:].bitcast(dt.int32), ALU.bitwise_or)
                        else:
                            # sign-merge as int add on the otherwise-idle
                            # Pool engine (fp16 magnitudes < 0x8000: no carry)
                            nc.gpsimd.tensor_tensor(
                                pw[:].bitcast(dt.int32),
                                sgn[:].bitcast(dt.int32),
                                sq[:].bitcast(dt.int32), ALU.add)
                        zt[wv] = (z16, a16, sq, pw)

                    def emit_acc(wv):
                        emit_accums((wv, zt[wv][3], zt[wv][1], numer, norm))
                        del zt[wv]

                    for wv in range(16):
                        g, part = wv // 4, wv % 4
                        if part == 0 and g + 2 < 4:
                            load_group(g + 2, b, i0, grp)
                        awt, bwt = grp[g]
                        zw = zp.tile([128, 4 * L], f32, tag="zw")
                        for sl in range(4):
                            k = part * 4 + sl
                            nc.tensor.matmul(
                                zw[:, sl * L:(sl + 1) * L],
                                bwt[:, k * 128:(k + 1) * 128],
                                awt[:, k * L:(k + 1) * L],
                                start=True, stop=True)
                        if wv >= 2:
                            emit_pw(wv - 2)
                        z16 = zsb2.tile([128, 4 * L], f16, tag="z16")
                        if wv % 16 in (0, 1, 3, 5, 6, 8, 10, 11, 13, 15):
                            nc.vector.tensor_copy(z16[:], zw[:])
                        else:
                            nc.scalar.activation(z16[:], zw[:], AF.Identity,
                                                 bias=0.0)
                        a16 = zsb.tile([128, 4 * L], f16, tag="a16")
                        nc.vector.tensor_scalar(
                            a16[:].bitcast(dt.int32), z16[:].bitcast(dt.int32),
                            0x7FFF7FFF, None, ALU.bitwise_and)
                        zt[wv] = (z16, a16)
                        if wv >= 1:
                            emit_sq(wv - 1)
                        if wv >= 4:
                            emit_acc(wv - 4)
                    emit_sq(15)
                    emit_pw(14)
                    emit_pw(15)
                    for wv in (12, 13, 14, 15):
                        emit_acc(wv)
                    sqn = zsbT.tile([128, L], f32, tag="sqn")
                    nc.scalar.activation(sqn[:], norm[:], AF.Sqrt, bias=1e-12)
                    rstd = zsbT.tile([128, L], f32, tag="rstd")
                    nc.vector.reciprocal(rstd[:], sqn[:])
                    contrib = zsbT.tile([128, L], f16, tag="contrib")
                    nc.vector.scalar_tensor_tensor(
                        contrib[:], numer[:], 1.0, rstd[:], ALU.mult, ALU.mult)
                    nc.sync.dma_start(contribs[i0:i0 + 128, :], contrib[:])

        # ---------- ReduceScatter scores (f16) ----------
        scores_d = dram.tile([S, L], f16, tag="scores")
        nc.gpsimd.collective_compute("ReduceScatter", ALU.add,
                                     ins=[contribs.opt()],
                                     outs=[scores_d.opt()],
                                     replica_groups=RG)

        # ---------- value projection (overlaps the RS chain) ----------
        # value tiles are copied out of PSUM directly in h-column e-order
        # (evens then odds) so they serve as the attend rhs per batch and
        # the scramble gather stays contiguous.
        atted_d = dram.tile([S, H], f16, tag="atted")
        with tc.tile_pool(name=f"l{L_}aps", bufs=2, space="PSUM") as aps:
            vsel = []
            value = []
            for mt in range(4):
                p = aps.tile([128, H], f32, tag="vproj")
                for kt in range(2):
                    nc.tensor.matmul(p[:], vT[kt][:, mt * 128:(mt + 1) * 128],
                                     w["Wv"][kt][:], start=(kt == 0), stop=False)
                nc.tensor.matmul(p[:], w["ones1"][:, 0:128], w["bvr"][:],
                                 start=False, stop=True)
                t = sb.tile([128, H], f32r, tag=f"value{mt}")
                nc.vector.tensor_copy(t[:], p[:])
                value.append(t)

            # value rows for this core's batch, h-columns permuted to
            # e-order (evens then odds) so the scramble gather is contiguous
            vsel = []
            for jt in range(2):
                p = aps.tile([128, H], f32, tag="vsel")
                for kt in range(4):
                    nc.tensor.matmul(p[:],
                                     w["sel"][kt][:, jt * 128:(jt + 1) * 128],
                                     value[kt][:],
                                     start=(kt == 0), stop=(kt == 3))
                t = sb.tile([128, H], f32r, tag=f"vsel{jt}")
                nc.vector.tensor_copy(
                    t[:].rearrange("j (ih iq) -> j ih iq", ih=2),
                    p[:].rearrange("j (iq ih) -> j ih iq", ih=2))
                vsel.append(t)

            # ---------- softmax + attend ----------
            # scores are bounded (|s| < ~30) so exp cannot overflow fp32:
            # skip the max-subtraction for a shorter serial chain.
            sc16 = sb.tile([S, L], f16, tag="sc16")
            nc.sync.dma_start(sc16[:], scores_d[:])
            # attend on unnormalized exp scores; the softmax denominator is
            # folded into the atted PSUM-exit as a per-row ACT scale.
            ex = sb.tile([S, L], f32, tag="ex")
            sume = sb.tile([S, 1], f32, tag="sume")
            nc.scalar.activation(ex[:], sc16[:], AF.Exp, bias=0.0,
                                 accum_out=sume[:])
            rcp = sb.tile([S, 1], f32, tag="rcp")
            nc.vector.reciprocal(rcp[:], sume[:])

            attT = []
            for jt in range(2):
                p = aps.tile([128, S], f32, tag="attT")
                nc.tensor.transpose(p[:], ex[:, jt * 128:(jt + 1) * 128],
                                    w["eye32"][0:S, 0:S])
                t = sb.tile([128, S], f32r, tag=f"attT{jt}")
                nc.vector.tensor_copy(t[:], p[:])
                attT.append(t)

            pa = aps.tile([S, H], f32, tag="atted")
            for jt in range(2):
                nc.tensor.matmul(pa[:], attT[jt][:], vsel[jt][:],
                                 start=(jt == 0), stop=(jt == 1))
            ats = sb.tile([S, H], f16, tag="atteds")
            nc.scalar.activation(ats[:], pa[:], AF.Identity, bias=0.0,
                                 scale=rcp[:])
            nc.sync.dma_start(atted_d[:], ats[:])

        # ---------- AllGather (f16 payload); scramble; Wm; BN ----------
        attedall = dram.tile([BL, H], f16, tag="attedall")
        nc.gpsimd.collective_compute("AllGather", ALU.bypass,
                                     ins=[atted_d.opt()],
                                     outs=[attedall.opt()],
                                     replica_groups=RG)

        with tc.tile_pool(name=f"l{L_}fps", bufs=1, space="PSUM") as fps:
            # scrT[j', (b,i')] = atted_perm[b*256 + j'%256, (j'//256)*256 + i']
            aa = attedall[:].rearrange("(b lh p) (ih iq) -> lh ih p b iq",
                                       b=2, lh=2, ih=2)
            scrT = []
            for jt in range(4):
                t = sb.tile([128, BL], f16, tag=f"scrT{jt}")
                nc.sync.dma_start(
                    t[:].rearrange("p (b iq) -> p b iq", b=2),
                    aa[jt % 2, jt // 2])
                scrT.append(t)

            attnT = []
            for mt in range(2):
                p = fps.tile([128, BL], f32, tag="attnT")
                for kt in range(4):
                    nc.tensor.matmul(p[:],
                                     w["Wm"][kt][:, mt * 128:(mt + 1) * 128],
                                     scrT[kt][:],
                                     start=(kt == 0), stop=(kt == 3))
                t = sb.tile([128, BL], f32r, tag=f"attnT{mt}")
                nc.scalar.activation(t[:], p[:], AF.Identity,
                                     bias=w["bmp"][:, mt:mt + 1])
                attnT.append(t)

            cs = fps.tile([1, BL], f32, tag="cs")
            for mt in range(2):
                nc.tensor.matmul(cs[:], w["ones128"][:], attnT[mt][:],
                                 start=(mt == 0), stop=(mt == 1))
            cs2 = fps.tile([1, BL], f32, tag="cs2")
            sqs = []
            for mt in range(2):
                sq = sb.tile([128, BL], f32r, tag=f"sq{mt}")
                nc.scalar.activation(sq[:], attnT[mt][:].bitcast(f32),
                                     AF.Square)
                sqs.append(sq)
            for mt in range(2):
                nc.tensor.matmul(cs2[:], w["ones128"][:], sqs[mt][:],
                                 start=(mt == 0), stop=(mt == 1))

            def stt(out, in0, scalar, in1, op0, op1):
                nc.vector.scalar_tensor_tensor(out, in0, scalar, in1, op0, op1)

            # per-position sums: fold the two batch halves (src0/src1 cannot
            # both be PSUM, so stage one half through SBUF); 1/BL folds into
            # the mu scale and the Sqrt input scale.
            css = sb.tile([1, BL], f32, tag="css")
            nc.vector.tensor_copy(css[:], cs[:])
            cs2s = sb.tile([1, BL], f32, tag="cs2s")
            nc.vector.tensor_copy(cs2s[:], cs2[:])
            csum = sb.tile([1, D], f32, tag="csum")
            nc.vector.tensor_tensor(csum[:], css[:, 0:D], css[:, D:BL],
                                    ALU.add)
            c2sum = sb.tile([1, D], f32, tag="c2sum")
            nc.vector.tensor_tensor(c2sum[:], cs2s[:, 0:D], cs2s[:, D:BL],
                                    ALU.add)
            mu = sb.tile([1, D], f32, tag="mu")
            nc.vector.tensor_scalar_mul(mu[:], csum[:], 1.0 / BL)
            varBL = sb.tile([1, D], f32, tag="varBL")
            stt(varBL[:], mu[:], -1.0, csum[:], ALU.mult, ALU.mult)
            stt(varBL[:], varBL[:], 1.0, c2sum[:], ALU.mult, ALU.add)
            sqv = sb.tile([1, D], f32, tag="sqv")
            nc.scalar.activation(sqv[:], varBL[:], AF.Sqrt, bias=1e-5,
                                 scale=1.0 / BL)
            rstd = sb.tile([1, D], f32, tag="rstdb")
            nc.vector.reciprocal(rstd[:], sqv[:])
            Am = sb.tile([1, D], f32, tag="Am")
            stt(Am[:], w["gam"][:], 1.0, rstd[:], ALU.mult, ALU.mult)
            Bb = sb.tile([1, D], f32, tag="Bb")
            stt(Bb[:], mu[:], -1.0, Am[:], ALU.mult, ALU.mult)
            stt(Bb[:], Bb[:], 1.0, w["bet"][:], ALU.mult, ALU.add)
            A2 = sb.tile([1, BL], f32r, tag="A2")
            nc.vector.tensor_copy(A2[:, 0:D], Am[:])
            nc.vector.tensor_copy(A2[:, D:BL], Am[:])
            B2 = sb.tile([1, BL], f32r, tag="B2")
            nc.vector.tensor_copy(B2[:, 0:D], Bb[:])
            nc.vector.tensor_copy(B2[:, D:BL], Bb[:])
            Abc = fps.tile([128, BL], f32, tag="Abc")
            nc.tensor.matmul(Abc[:], w["ones1"][:, 0:128], A2[:],
                             start=True, stop=True)
            Bbc = fps.tile([128, BL], f32, tag="Bbc")
            nc.tensor.matmul(Bbc[:], w["ones1"][:, 0:128], B2[:],
                             start=True, stop=True)

            vnew = []
            for mt in range(2):
                t1 = sb.tile([128, BL], f32, tag="t1")
                stt(t1[:], attnT[mt][:].bitcast(f32), 1.0, Abc[:],
                    ALU.mult, ALU.mult)
                stt(t1[:], t1[:], 1.0, Bbc[:], ALU.mult, ALU.add)
                t2 = w["cp"].tile([128, BL], f32r, tag=f"vnew{L_}{mt}")
                stt(t2[:], t1[:], 0.0, vT[mt][:].bitcast(f32), ALU.max,
                    ALU.add)
                vnew.append(t2)
            return vnew


def _prep_inputs(inp, core):
    g = lambda k: np.asarray(inp[k], dtype=np.float32)
    xi, xt = g("xi"), g("xt")
    xiT = np.ascontiguousarray(xi.transpose(2, 0, 1).reshape(D, BL))
    xtT = np.ascontiguousarray(xt.transpose(2, 0, 1).reshape(D, BL))
    c = core
    W0c = np.ascontiguousarray(g("W0")[:, c * S:(c + 1) * S])
    W1c = np.ascontiguousarray(g("W1")[:, c * S:(c + 1) * S])

    def padM(M):  # (S, S*R) -> (S, S*RP), col order s*RP+r
        Mr = M.reshape(S, R, S).transpose(0, 2, 1)
        Mp = np.zeros((S, S, RP), np.float32)
        Mp[:, :, :R] = Mr
        return np.ascontiguousarray(Mp.reshape(S, S * RP))

    def padMb(Mb):  # (S*R,) -> per-partition (128, S*RP//128)
        Mbr = Mb.reshape(R, S).T
        Mbp = np.zeros((S, RP), np.float32)
        Mbp[:, :R] = Mbr
        return np.ascontiguousarray(Mbp.reshape(S * RP // 128, 128).T)

    bcore = c // 4
    selm = np.zeros((BL, L), np.float32)
    selm[np.arange(L) + bcore * L, np.arange(L)] = 1.0

    wc = g("Wout")[c * S:(c + 1) * S, 0]
    eye = np.eye(128, dtype=np.float16)
    eyeW = np.ascontiguousarray(
        (wc.astype(np.float16)[:, None, None] * eye)
        .transpose(1, 0, 2).reshape(128, S * 128))

    return {
        "xiT": xiT, "xtT": xtT,
        "Wv": g("Wv"), "bvr": np.ascontiguousarray(g("bv").reshape(1, H)),
        "Wk": g("Wk"), "bkp": np.ascontiguousarray(g("bk").reshape(4, 128).T),
        "Wq": g("Wq"), "bqp": np.ascontiguousarray(g("bq").reshape(4, 128).T),
        "Wm": g("Wm").astype(np.float16), "bmp": np.ascontiguousarray(g("bm").reshape(2, 128).T),
        "W0c": W0c, "b0c": np.ascontiguousarray(
            g("b0")[c * S:(c + 1) * S].reshape(S, 1)),
        "W1c": W1c, "b1c": np.ascontiguousarray(
            g("b1")[c * S:(c + 1) * S].reshape(S, 1)),
        "M0p": padM(g("M0")[c]), "Mb0p": padMb(g("Mb0")[c]),
        "M1p": padM(g("M1")[c]), "Mb1p": padMb(g("Mb1")[c]),
        "eyeW": eyeW,
        "eye16": np.eye(128, dtype=np.float16),
        "eye32": np.eye(128, dtype=np.float32),
        "ones1": np.ones((1, 128), np.float32),
        "ones128": np.ones((128, 1), np.float32),
        "sel": selm,
        "gam": np.ascontiguousarray(g("gamma").reshape(1, D)),
        "bet": np.ascontiguousarray(g("beta").reshape(1, D)),
    }


def kernel(**inputs):
    if "nc" not in _cache:
        _cache["nc"] = build_bass()
    nc = _cache["nc"]
    in_maps = [_prep_inputs(inputs, c) for c in range(NCORES)]
    res = run_bass_kernel_spmd(nc, in_maps, core_ids=list(range(NCORES)))
    vT = np.asarray(res.results[0]["vT_out"], dtype=np.float32)
    out = vT.reshape(D, B, L).transpose(1, 2, 0)
    return np.ascontiguousarray(out)
